# revision 15
# baseline (speedup 1.0000x reference)
"""Trainium2 Bass kernel for multi-head self-attention (no causal mask).

Reference computation (B=2, T=2048, C=1024, H=16, hd=64):
    qkv = x @ w_attn.T + b_attn                     # [B,T,3C], per-head interleaved
    q,k,v split per head (head h owns rows h*192 .. h*192+191 of w_attn:
        +0..63 = q, +64..127 = k, +128..191 = v)
    attn = softmax(q @ k.T * C**-0.5)               # NOTE scale uses C, no mask
    y = attn @ v -> [B,T,C] -> y @ w_proj.T + b_proj

Sharding (8 cores, tensor parallel over heads):
  - core i computes qkv^T for heads 2i,2i+1 (w_attn rows [384i, 384(i+1)))
    over all tokens, in transposed layout [384, B*T]. The per-core weight
    slice is column-permuted host-side so SBUF tile0 holds Q^T (head0 at
    partitions 0:64, head1 at 64:128), tile1 K^T, tile2 V^T — this keeps
    every matmul's lhsT/rhs at the same base partition (PE quadrant rule).
  - attention per (b, head) with scores computed directly transposed
    (S^T[kt,qt]) so the PV matmul needs no transpose of P. Softmax uses
    exp without max subtraction (scores are ~N(0, 0.25^2), max |S| < 2) and
    gets denominators for free from a ones-column appended to V.
  - emission is software-pipelined: batch-1 qkv matmuls are interleaved
    into batch-0 attention chunk units (the per-engine instruction streams
    are in-order, so emission order controls overlap), and the first half
    of the output projection is interleaved into batch-1 attention.
  - TWO AllToAll collectives (one per batch) exchange per-head outputs;
    core j ends with full-C y^T for 256 tokens of b0 plus 256 tokens of b1
    and projects them with full w_proj. The first collective is hidden
    behind batch-1 attention compute.
  - host reassembles the 8 [512, 1024] output shards (rows 0:256 = b0
    tokens j*256.., rows 256:512 = b1 tokens j*256..).

All matmuls run as float32r (TF32-like 1 cycle/row mode; plain fp32 is 4
cycles/row). fp32r requires every producer feeding a matmul to emit
fp32r-typed output (walrus verifier rule). DVE lanes cannot cross
partitions and the gpsimd partition_broadcast ucode ignores the AP base
partition on HW, so the softmax-reciprocal and bias broadcasts use K=1 PE
matmuls against a ones vector.
"""

import os
import numpy as np

import concourse.bass as bass
import concourse.bacc as bacc
import concourse.mybir as mybir
import concourse.tile as tile
from concourse.bass_utils import run_bass_kernel_spmd

N_CORES = 8
B, T, C = 2, 2048, 1024
H, HD = 16, 64
HPC = H // N_CORES          # heads per core = 2
BT = B * T                  # 4096 flattened tokens
OSL = HPC * 3 * HD          # 384 qkv rows per core
TSL = BT // N_CORES         # 512 output tokens per core
TSH = TSL // 2              # 256 tokens per batch per core
TCH = 512                   # token chunk for qkv matmul
QCH = 512                   # query chunk in attention
SCALE = float(C) ** -0.5    # softmax scale (uses C, faithful to reference)

FP32 = mybir.dt.float32
F32R = mybir.dt.float32r

NC_T = BT // TCH   # 8 token chunks
NC_C = C // 128    # 8 contraction tiles
NKT = T // 128     # 16 key tiles per batch
NQC = T // QCH     # 4 query chunks per batch


def build_program(single_core=False):
    nc = bacc.Bacc(
        "TRN2",
        target_bir_lowering=False,
        debug=False,
        enable_asserts=False,
        num_devices=1 if single_core else N_CORES,
    )

    xT = nc.dram_tensor("xT", [C, BT], F32R, kind="ExternalInput")
    wqT = nc.dram_tensor("wqT", [C, OSL], F32R, kind="ExternalInput")
    bq = nc.dram_tensor("bq", [OSL, 1], FP32, kind="ExternalInput")
    wpT = nc.dram_tensor("wpT", [C, C], F32R, kind="ExternalInput")
    bp = nc.dram_tensor("bp", [1, C], F32R, kind="ExternalInput")
    ident = nc.dram_tensor("ident", [128, HD], F32R, kind="ExternalInput")
    ones128 = nc.dram_tensor("ones128", [128, 128], F32R, kind="ExternalInput")
    out = nc.dram_tensor("out", [TSL, C], FP32, kind="ExternalOutput")

    with tile.TileContext(nc) as tc:
        with (
            tc.tile_pool(name="persist", bufs=1) as persist,
            tc.tile_pool(name="dram", bufs=1, space="DRAM") as dram,
            tc.tile_pool(name="xin", bufs=2) as xin,
            tc.tile_pool(name="ps_qkv", bufs=2, space="PSUM") as ps_qkv,
            tc.tile_pool(name="ps_sp", bufs=2, space="PSUM") as ps_sp,
            tc.tile_pool(name="ps_y", bufs=2, space="PSUM") as ps_y,
            tc.tile_pool(name="ps_o", bufs=2, space="PSUM") as ps_o,
            tc.tile_pool(name="pbuf", bufs=3) as pbuf,
            tc.tile_pool(name="vbuf", bufs=3) as vbuf,
            tc.tile_pool(name="small", bufs=2) as small,
            tc.tile_pool(name="ybuf", bufs=2) as ybuf,
            tc.tile_pool(name="ytbuf", bufs=1) as ytbuf,
            tc.tile_pool(name="obuf", bufs=2) as obuf,
        ):
            # ---- persistent SBUF state (wp/bp loads deferred: keep the
            # DMA queue head free for the x chunks feeding the first qkv) ----
            id_sb = persist.tile([128, HD], F32R)
            nc.sync.dma_start(id_sb, ident.ap())
            ones_sb = persist.tile([128, 128], F32R)
            nc.sync.dma_start(ones_sb, ones128.ap())
            wq_sb = persist.tile([128, NC_C, OSL], F32R)
            nc.sync.dma_start(wq_sb, wqT.rearrange("(n p) f -> p n f", p=128))
            bq_sb = persist.tile([128, OSL // 128, 1], FP32)
            nc.sync.dma_start(bq_sb, bq.rearrange("(n p) o -> p n o", p=128))
            wp_sb = persist.tile([128, NC_C, C], F32R)
            bp_sb = persist.tile([1, C], F32R)

            # qkv^T, permuted layout: o-tile 0 = Q^T, 1 = K^T, 2 = V^T;
            # head0 at partitions 0:64, head1 at 64:128
            qkvT = persist.tile([128, 3, BT], F32R)
            bb = persist.tile([128, C], FP32)

            a2a_in = [dram.tile([N_CORES, 128, TSH], F32R, name=f"a2ai{b}")
                      for b in range(B)]
            a2a_out = [dram.tile([N_CORES, 128, TSH], F32R, name=f"a2ao{b}")
                       for b in range(B)]
            yT_sb = ytbuf.tile([128, NC_C, TSL], F32R)

            # ---- emission helpers ----
            def emit_x_dma(tci):
                sl = slice(tci * TCH, (tci + 1) * TCH)
                xc = xin.tile([128, NC_C, TCH], F32R, tag="xc", name=f"xc{tci}")
                nc.sync.dma_start(
                    xc, xT[:, sl].rearrange("(n p) f -> p n f", p=128))
                return xc

            def emit_qkv_group(xc, tci, ot):
                sl = slice(tci * TCH, (tci + 1) * TCH)
                ps = ps_qkv.tile([128, TCH], FP32, tag="ps", name=f"qk{tci}{ot}")
                for ct in range(NC_C):
                    nc.tensor.matmul(
                        ps, lhsT=wq_sb[:, ct, ot * 128:(ot + 1) * 128],
                        rhs=xc[:, ct, :], start=(ct == 0), stop=(ct == NC_C - 1))
                nc.vector.tensor_scalar_add(qkvT[:, ot, sl], ps, bq_sb[:, ot, :])

            def emit_va(b, hl):
                hp = hl * HD
                boff = b * T
                v_ap = qkvT[hp:hp + HD, 2, boff:boff + T]
                va = vbuf.tile([128, NKT, HD + 1], F32R, tag="va",
                               name=f"va{b}{hl}")
                nc.sync.dma_start(va[:, :, HD], ones128.ap()[:, 0:NKT])
                for k in range(NKT):
                    tp = ps_qkv.tile([128, HD], F32R, tag="ps", name=f"tp{k}")
                    nc.tensor.transpose(
                        tp, in_=v_ap[:, k * 128:(k + 1) * 128],
                        identity=id_sb[hp:hp + HD, :])
                    nc.vector.tensor_copy(va[:, k, 0:HD], tp)
                return va

            def emit_chunk(b, hl, qc, va, prev_finish=None):
                """One attention chunk unit: S^T/exp/PV over all kt for one
                512-token query chunk. Normalization+staging is returned as
                a closure so the caller can defer it into the next chunk's
                kt loop (keeps the PE stream from stalling on the DVE
                reciprocal at chunk boundaries)."""
                hp = hl * HD
                boff = b * T
                q_ap = qkvT[hp:hp + HD, 0, boff:boff + T]
                k_ap = qkvT[hp:hp + HD, 1, boff:boff + T]
                qsl = slice(qc * QCH, (qc + 1) * QCH)
                ypx = ps_y.tile([128, QCH], FP32, tag="ypx", name=f"y{b}{hl}{qc}")
                for k in range(NKT):
                    sps = ps_sp.tile([128, QCH], FP32, tag="sps", name=f"s{k}")
                    nc.tensor.matmul(
                        sps, lhsT=k_ap[:, k * 128:(k + 1) * 128],
                        rhs=q_ap[:, qsl], start=True, stop=True)
                    pt = pbuf.tile([128, QCH], F32R, tag="pt", name=f"p{k}")
                    nc.scalar.activation(
                        pt, sps, mybir.ActivationFunctionType.Exp, scale=SCALE)
                    nc.tensor.matmul(
                        ypx[0:HD + 1, :], lhsT=va[:, k, :], rhs=pt,
                        start=(k == 0), stop=(k == NKT - 1))
                    if k == 1 and prev_finish is not None:
                        prev_finish()
                        prev_finish = None
                if prev_finish is not None:
                    prev_finish()

                def finish():
                    # normalize by the denominator accumulated in partition HD
                    rec = small.tile([HD + 1, QCH], F32R, tag="rec", name="rec")
                    with nc.allow_low_precision(reason="f32r recip"):
                        nc.vector.reciprocal(rec[HD:HD + 1, :],
                                             ypx[HD:HD + 1, :])
                    rbp = ps_sp.tile([HD, QCH], FP32, tag="sps", name="rbp")
                    nc.tensor.matmul(rbp, lhsT=ones_sb[HD:HD + 1, 0:HD],
                                     rhs=rec[HD:HD + 1, :],
                                     start=True, stop=True)
                    rb = small.tile([HD, QCH], FP32, tag="rb", name="rb")
                    nc.vector.tensor_copy(rb, rbp)
                    yc = ybuf.tile([HD, QCH], F32R, tag="yc", name="yc")
                    nc.vector.tensor_tensor(yc, ypx[0:HD, :], rb,
                                            op=mybir.AluOpType.mult)
                    # stage into the per-batch a2a input: dest core j holds
                    # batch tokens [j*TSH, (j+1)*TSH); this 512-token chunk
                    # covers dests 2qc and 2qc+1
                    for half in range(2):
                        j = 2 * qc + half
                        nc.sync.dma_start(
                            a2a_in[b][j, hp:hp + HD, :],
                            yc[:, half * TSH:(half + 1) * TSH])
                return finish

            def emit_a2a(b):
                if single_core:
                    nc.sync.dma_start(a2a_out[b].opt(), a2a_in[b].opt())
                else:
                    nc.gpsimd.collective_compute(
                        "AllToAll", mybir.AluOpType.bypass,
                        replica_groups=[list(range(N_CORES))],
                        ins=[a2a_in[b].opt()], outs=[a2a_out[b].opt()])

            def emit_yt_load(b):
                for ct in range(NC_C):
                    nc.sync.dma_start(
                        yT_sb[:, ct, b * TSH:(b + 1) * TSH], a2a_out[b][ct])

            def emit_proj(tt):
                osb = obuf.tile([128, C], FP32, tag="osb", name=f"o{tt}")
                for oc in range(C // 512):
                    osl = slice(oc * 512, (oc + 1) * 512)
                    ps = ps_o.tile([128, 512], FP32, tag="ops", name=f"op{tt}{oc}")
                    for ct in range(NC_C):
                        nc.tensor.matmul(
                            ps, lhsT=yT_sb[:, ct, tt * 128:(tt + 1) * 128],
                            rhs=wp_sb[:, ct, osl],
                            start=(ct == 0), stop=(ct == NC_C - 1))
                    nc.vector.tensor_add(osb[:, osl], ps, bb[:, osl])
                nc.sync.dma_start(out[tt * 128:(tt + 1) * 128, :], osb)

            # ---- emission schedule (software pipeline) ----
            def emit_wp_load():
                nc.sync.dma_start(
                    wp_sb, wpT.rearrange("(n p) f -> p n f", p=128))
                nc.sync.dma_start(bp_sb, bp.ap())

            def emit_bias_bcast():
                for oc in range(C // 512):
                    osl = slice(oc * 512, (oc + 1) * 512)
                    bbp = ps_o.tile([128, 512], FP32, tag="ops",
                                    name=f"bbp{oc}")
                    nc.tensor.matmul(bbp, lhsT=ones_sb[0:1, :],
                                     rhs=bp_sb[:, osl], start=True, stop=True)
                    nc.vector.tensor_copy(bb[:, osl], bbp)

            # PE prewarm: ~4us of dummy matmul activity releases the HAM
            # clock-gate before the first real qkv matmul arrives
            warm = ps_sp.tile([128, 128], FP32, tag="sps", name="warm")
            for wi in range(18):
                nc.tensor.matmul(warm, lhsT=ones_sb, rhs=ones_sb,
                                 start=(wi == 0), stop=(wi == 17))

            # batch-0 qkv
            for tci in range(4):
                xc = emit_x_dma(tci)
                for ot in range(3):
                    emit_qkv_group(xc, tci, ot)

            vas = {}
            vas[(0, 0)] = emit_va(0, 0)
            vas[(0, 1)] = emit_va(0, 1)

            # filler work interleaved into batch-0 attention chunk units:
            # batch-1 qkv, deferred weight loads, batch-1 V_aug builds
            state = {"xc": None}
            # (pe_cost, fn): DMA-only items cost 0 and don't consume a slot
            fillers = []
            for tci in range(4, 8):
                fillers.append((0, lambda t=tci: state.update(
                    xc=emit_x_dma(t))))
                for ot in range(3):
                    fillers.append((1, lambda t=tci, o=ot: emit_qkv_group(
                        state["xc"], t, o)))
            fillers.append((0, emit_wp_load))
            fillers.append((1, lambda: vas.update({(1, 0): emit_va(1, 0)})))
            fillers.append((1, lambda: vas.update({(1, 1): emit_va(1, 1)})))
            fillers.append((1, emit_bias_bcast))

            def pop_fillers(n):
                budget = n
                while fillers and budget > 0:
                    cost, f = fillers.pop(0)
                    f()
                    budget -= cost

            pending = None
            for hl in range(HPC):
                for qc in range(NQC):
                    pending = emit_chunk(0, hl, qc, vas[(0, hl)], pending)
                    pop_fillers(2)
            while fillers:
                fillers.pop(0)[1]()
            pending()
            pending = None
            emit_a2a(0)

            # batch-1 attention with first-half projection interleaved
            for hl in range(HPC):
                for qc in range(NQC):
                    pending = emit_chunk(1, hl, qc, vas[(1, hl)], pending)
                    if hl == 1 and qc == 0:
                        emit_yt_load(0)
                    elif hl == 1 and qc == 1:
                        emit_proj(0)
                    elif hl == 1 and qc == 2:
                        emit_proj(1)
            pending()
            emit_a2a(1)
            emit_yt_load(1)
            emit_proj(2)
            emit_proj(3)

    nc.compile()
    return nc


_NC_CACHE = None


def _get_program():
    global _NC_CACHE
    if _NC_CACHE is None:
        _NC_CACHE = build_program()
    return _NC_CACHE


# permutation of the 384 local qkv rows: tile0 = [q_h0, q_h1],
# tile1 = [k_h0, k_h1], tile2 = [v_h0, v_h1]
def _local_perm():
    p = []
    for kind in range(3):            # q, k, v
        for hl in range(HPC):
            base = hl * 3 * HD + kind * HD
            p.extend(range(base, base + HD))
    return np.array(p, dtype=np.int64)


def make_in_maps(x, w_attn, b_attn, w_proj, b_proj):
    x = np.asarray(x, dtype=np.float32)
    w_attn = np.asarray(w_attn, dtype=np.float32)
    b_attn = np.asarray(b_attn, dtype=np.float32)
    w_proj = np.asarray(w_proj, dtype=np.float32)
    b_proj = np.asarray(b_proj, dtype=np.float32)

    xT = np.ascontiguousarray(x.reshape(BT, C).T)
    wpT = np.ascontiguousarray(w_proj.T)
    bp = np.ascontiguousarray(b_proj.reshape(1, C))
    ident = np.concatenate([np.eye(HD, dtype=np.float32)] * 2, axis=0)
    ones128 = np.ones((128, 128), dtype=np.float32)
    perm = _local_perm()

    in_maps = []
    for i in range(N_CORES):
        sl = slice(i * OSL, (i + 1) * OSL)
        w_slice = w_attn[sl][perm]
        b_slice = b_attn[sl][perm]
        in_maps.append({
            "xT": xT,
            "wqT": np.ascontiguousarray(w_slice.T),
            "bq": np.ascontiguousarray(b_slice.reshape(OSL, 1)),
            "wpT": wpT,
            "bp": bp,
            "ident": ident,
            "ones128": ones128,
        })
    return in_maps


def _assemble(shards):
    """shards[j] is [TSL, C]: rows 0:TSH = b0 tokens j*TSH.., rows
    TSH:2*TSH = b1 tokens j*TSH.. Reassemble into [B, T, C]."""
    out = np.empty((BT, C), dtype=np.float32)
    for j in range(N_CORES):
        out[j * TSH:(j + 1) * TSH] = shards[j][0:TSH]
        out[T + j * TSH:T + (j + 1) * TSH] = shards[j][TSH:2 * TSH]
    return out.reshape(B, T, C)


def kernel(x, w_attn, b_attn, w_proj, b_proj):
    nc = _get_program()
    in_maps = make_in_maps(x, w_attn, b_attn, w_proj, b_proj)

    if os.environ.get("BASS_KERNEL_SIM") == "1":
        from concourse.bass_interp import MultiCoreSim
        sim = MultiCoreSim(nc, num_cores=N_CORES,
                           num_workers=int(os.environ.get("BASS_SIM_WORKERS", "8")))
        for i in range(N_CORES):
            core = sim.cores[i]
            for k, v in in_maps[i].items():
                core.tensor(k)[:] = v
        sim.simulate(check_with_hw=False)
        shards = [np.array(sim.cores[i].tensor("out")) for i in range(N_CORES)]
    else:
        last_err = None
        shards = None
        for _attempt in range(3):
            try:
                res = run_bass_kernel_spmd(nc, in_maps,
                                           core_ids=list(range(N_CORES)))
                shards = [res.results[i]["out"] for i in range(N_CORES)]
                break
            except Exception as e:  # transient device wedge: retry
                last_err = e
        if shards is None:
            raise last_err

    return _assemble(shards)


# revision 18
# speedup vs baseline: 4.1526x; 4.1526x over previous
"""Trainium2 Bass kernel for multi-head self-attention (no causal mask).

Reference computation (B=2, T=2048, C=1024, H=16, hd=64):
    qkv = x @ w_attn.T + b_attn                     # [B,T,3C], per-head interleaved
    q,k,v split per head (head h owns rows h*192 .. h*192+191 of w_attn:
        +0..63 = q, +64..127 = k, +128..191 = v)
    attn = softmax(q @ k.T * C**-0.5)               # NOTE scale uses C, no mask
    y = attn @ v -> [B,T,C] -> y @ w_proj.T + b_proj

Sharding (8 cores, tensor parallel over heads):
  - core i computes qkv^T for heads 2i,2i+1 (w_attn rows [384i, 384(i+1)))
    over all tokens, in transposed layout [384, B*T]. The per-core weight
    slice is column-permuted host-side so SBUF tile0 holds Q^T (head0 at
    partitions 0:64, head1 at 64:128), tile1 K^T, tile2 V^T — this keeps
    every matmul's lhsT/rhs at the same base partition (PE quadrant rule).
  - attention per (b, head) with scores computed directly transposed
    (S^T[kt,qt]) so the PV matmul needs no transpose of P. Softmax uses
    exp without max subtraction (scores are ~N(0, 0.25^2), max |S| < 2) and
    gets denominators for free from a ones-column appended to V.
  - emission is software-pipelined: batch-1 qkv matmuls are interleaved
    into batch-0 attention chunk units (the per-engine instruction streams
    are in-order, so emission order controls overlap), and the first half
    of the output projection is interleaved into batch-1 attention.
  - TWO AllToAll collectives (one per batch) exchange per-head outputs;
    core j ends with full-C y^T for 256 tokens of b0 plus 256 tokens of b1
    and projects them with full w_proj. The first collective is hidden
    behind batch-1 attention compute.
  - host reassembles the 8 [512, 1024] output shards (rows 0:256 = b0
    tokens j*256.., rows 256:512 = b1 tokens j*256..).

All matmuls run as float32r (TF32-like 1 cycle/row mode; plain fp32 is 4
cycles/row). fp32r requires every producer feeding a matmul to emit
fp32r-typed output (walrus verifier rule). DVE lanes cannot cross
partitions and the gpsimd partition_broadcast ucode ignores the AP base
partition on HW, so the softmax-reciprocal and bias broadcasts use K=1 PE
matmuls against a ones vector.
"""

import os
import numpy as np

import concourse.bass as bass
import concourse.bacc as bacc
import concourse.mybir as mybir
import concourse.tile as tile
from concourse.bass_utils import run_bass_kernel_spmd

N_CORES = 8
B, T, C = 2, 2048, 1024
H, HD = 16, 64
HPC = H // N_CORES          # heads per core = 2
BT = B * T                  # 4096 flattened tokens
OSL = HPC * 3 * HD          # 384 qkv rows per core
TSL = BT // N_CORES         # 512 output tokens per core
TSH = TSL // 2              # 256 tokens per batch per core
TCH = 512                   # token chunk for qkv matmul
QCH = 512                   # query chunk in attention
SCALE = float(C) ** -0.5    # softmax scale (uses C, faithful to reference)

FP32 = mybir.dt.float32
F32R = mybir.dt.float32r

NC_T = BT // TCH   # 8 token chunks
NC_C = C // 128    # 8 contraction tiles
NKT = T // 128     # 16 key tiles per batch
NQC = T // QCH     # 4 query chunks per batch


def build_program(single_core=False, variant="full"):
    nc = bacc.Bacc(
        "TRN2",
        target_bir_lowering=False,
        debug=False,
        enable_asserts=False,
        num_devices=1 if single_core else N_CORES,
    )

    xT = nc.dram_tensor("xT", [C, BT], F32R, kind="ExternalInput")
    wqT = nc.dram_tensor("wqT", [C, OSL], F32R, kind="ExternalInput")
    bq = nc.dram_tensor("bq", [OSL, 1], FP32, kind="ExternalInput")
    wpT = nc.dram_tensor("wpT", [C, C], F32R, kind="ExternalInput")
    bp = nc.dram_tensor("bp", [1, C], F32R, kind="ExternalInput")
    ident = nc.dram_tensor("ident", [128, HD], F32R, kind="ExternalInput")
    ones128 = nc.dram_tensor("ones128", [128, 128], F32R, kind="ExternalInput")
    out = nc.dram_tensor("out", [TSL, C], FP32, kind="ExternalOutput")

    class _EndEmission(Exception):
        pass

    with tile.TileContext(nc) as tc:
      try:
        with (
            tc.tile_pool(name="persist", bufs=1) as persist,
            tc.tile_pool(name="dram", bufs=1, space="DRAM") as dram,
            tc.tile_pool(name="xin", bufs=2) as xin,
            tc.tile_pool(name="ps_qkv", bufs=2, space="PSUM") as ps_qkv,
            tc.tile_pool(name="ps_sp", bufs=2, space="PSUM") as ps_sp,
            tc.tile_pool(name="ps_y", bufs=2, space="PSUM") as ps_y,
            tc.tile_pool(name="pbuf", bufs=3) as pbuf,
            tc.tile_pool(name="vbuf", bufs=3) as vbuf,
            tc.tile_pool(name="small", bufs=2) as small,
            tc.tile_pool(name="ybuf", bufs=2) as ybuf,
            tc.tile_pool(name="ytbuf", bufs=1) as ytbuf,
            tc.tile_pool(name="obuf", bufs=2) as obuf,
        ):
            # ---- persistent SBUF state (wp/bp loads deferred: keep the
            # DMA queue head free for the x chunks feeding the first qkv) ----
            id_sb = persist.tile([128, HD], F32R)
            nc.sync.dma_start(id_sb, ident.ap())
            ones_sb = persist.tile([128, 128], F32R)
            nc.sync.dma_start(ones_sb, ones128.ap())
            wq_sb = persist.tile([128, NC_C, OSL], F32R)
            nc.sync.dma_start(wq_sb, wqT.rearrange("(n p) f -> p n f", p=128))
            bq_sb = persist.tile([128, OSL // 128, 1], FP32)
            nc.sync.dma_start(bq_sb, bq.rearrange("(n p) o -> p n o", p=128))
            wp_sb = persist.tile([128, NC_C, C], F32R)
            bp_sb = persist.tile([1, C], F32R)

            # qkv^T, permuted layout: o-tile 0 = Q^T, 1 = K^T, 2 = V^T;
            # head0 at partitions 0:64, head1 at 64:128
            qkvT = persist.tile([128, 3, BT], F32R)
            bb = persist.tile([128, C], FP32)

            a2a_in = [dram.tile([N_CORES, 128, TSH], F32R, name=f"a2ai{b}")
                      for b in range(B)]
            a2a_out = [dram.tile([N_CORES, 128, TSH], F32R, name=f"a2ao{b}")
                       for b in range(B)]
            yT_sb = ytbuf.tile([128, NC_C, TSL], F32R)

            # ---- emission helpers ----
            def emit_x_dma(tci):
                sl = slice(tci * TCH, (tci + 1) * TCH)
                xc = xin.tile([128, NC_C, TCH], F32R, tag="xc", name=f"xc{tci}")
                nc.sync.dma_start(
                    xc, xT[:, sl].rearrange("(n p) f -> p n f", p=128))
                return xc

            def emit_qkv_group(xc, tci, ot):
                sl = slice(tci * TCH, (tci + 1) * TCH)
                ps = ps_qkv.tile([128, TCH], FP32, tag="ps", name=f"qk{tci}{ot}")
                for ct in range(NC_C):
                    nc.tensor.matmul(
                        ps, lhsT=wq_sb[:, ct, ot * 128:(ot + 1) * 128],
                        rhs=xc[:, ct, :], start=(ct == 0), stop=(ct == NC_C - 1))
                nc.vector.tensor_scalar_add(qkvT[:, ot, sl], ps, bq_sb[:, ot, :])

            def emit_va(b, hl):
                hp = hl * HD
                boff = b * T
                v_ap = qkvT[hp:hp + HD, 2, boff:boff + T]
                va = vbuf.tile([128, NKT, HD + 1], F32R, tag="va",
                               name=f"va{b}{hl}")
                nc.sync.dma_start(va[:, :, HD], ones128.ap()[:, 0:NKT])
                for k in range(NKT):
                    tp = ps_qkv.tile([128, HD], F32R, tag="ps", name=f"tp{k}")
                    nc.tensor.transpose(
                        tp, in_=v_ap[:, k * 128:(k + 1) * 128],
                        identity=id_sb[hp:hp + HD, :])
                    nc.vector.tensor_copy(va[:, k, 0:HD], tp)
                return va

            def emit_chunk(b, hl, qp, va, prev_finish=None):
                """One attention pair unit: S^T/exp/PV over all kt for TWO
                512-token query chunks (one 1024-wide exp per kt tile).
                Normalization+staging is returned as a closure deferred into
                the next unit (keeps the PE stream from stalling on the DVE
                reciprocal at unit boundaries)."""
                hp = hl * HD
                boff = b * T
                q_ap = qkvT[hp:hp + HD, 0, boff:boff + T]
                k_ap = qkvT[hp:hp + HD, 1, boff:boff + T]
                W = 2 * QCH
                qsl = slice(qp * W, (qp + 1) * W)
                ypx = [ps_y.tile([128, QCH], FP32, tag="ypx",
                                 name=f"y{b}{hl}{qp}{h}") for h in range(2)]
                for k in range(NKT):
                    sps = ps_sp.tile([128, W], FP32, tag="sps", name=f"s{k}")
                    for h in range(2):
                        nc.tensor.matmul(
                            sps[:, h * QCH:(h + 1) * QCH],
                            lhsT=k_ap[:, k * 128:(k + 1) * 128],
                            rhs=q_ap[:, qp * W + h * QCH:
                                     qp * W + (h + 1) * QCH],
                            start=True, stop=True)
                    if k == 0 and prev_finish is not None:
                        prev_finish()
                        prev_finish = None
                    pt = pbuf.tile([128, W], F32R, tag="pt", name=f"p{k}")
                    nc.scalar.activation(
                        pt, sps, mybir.ActivationFunctionType.Exp, scale=SCALE)
                    for h in range(2):
                        nc.tensor.matmul(
                            ypx[h][0:HD + 1, :], lhsT=va[:, k, :],
                            rhs=pt[:, h * QCH:(h + 1) * QCH],
                            start=(k == 0), stop=(k == NKT - 1))
                if prev_finish is not None:
                    prev_finish()

                def finish():
                    # normalize by the denominators accumulated in
                    # partition HD of each accumulator
                    for h in range(2):
                        qc = 2 * qp + h
                        rec = small.tile([HD + 1, QCH], F32R, tag="rec",
                                         name="rec")
                        with nc.allow_low_precision(reason="f32r recip"):
                            nc.vector.reciprocal(rec[HD:HD + 1, :],
                                                 ypx[h][HD:HD + 1, :])
                        rbp = ps_sp.tile([HD, QCH], FP32, tag="sps",
                                         name="rbp")
                        nc.tensor.matmul(rbp, lhsT=ones_sb[HD:HD + 1, 0:HD],
                                         rhs=rec[HD:HD + 1, :],
                                         start=True, stop=True)
                        rb = small.tile([HD, QCH], FP32, tag="rb", name="rb")
                        nc.vector.tensor_copy(rb, rbp)
                        yc = ybuf.tile([HD, QCH], F32R, tag="yc", name="yc")
                        nc.vector.tensor_tensor(yc, ypx[h][0:HD, :], rb,
                                                op=mybir.AluOpType.mult)
                        # dest core j holds batch tokens [j*TSH, (j+1)*TSH)
                        for half in range(2):
                            j = 2 * qc + half
                            nc.sync.dma_start(
                                a2a_in[b][j, hp:hp + HD, :],
                                yc[:, half * TSH:(half + 1) * TSH])
                return finish

            def emit_a2a(b):
                if single_core or variant == "noccl":
                    nc.sync.dma_start(a2a_out[b].opt(), a2a_in[b].opt())
                else:
                    nc.gpsimd.collective_compute(
                        "AllToAll", mybir.AluOpType.bypass,
                        replica_groups=[list(range(N_CORES))],
                        ins=[a2a_in[b].opt()], outs=[a2a_out[b].opt()])

            def emit_yt_load(b):
                for ct in range(NC_C):
                    nc.sync.dma_start(
                        yT_sb[:, ct, b * TSH:(b + 1) * TSH], a2a_out[b][ct])

            def emit_proj(tt):
                osb = obuf.tile([128, C], FP32, tag="osb", name=f"o{tt}")
                for oc in range(C // 512):
                    osl = slice(oc * 512, (oc + 1) * 512)
                    ps = ps_qkv.tile([128, 512], FP32, tag="ps", name=f"op{tt}{oc}")
                    for ct in range(NC_C):
                        nc.tensor.matmul(
                            ps, lhsT=yT_sb[:, ct, tt * 128:(tt + 1) * 128],
                            rhs=wp_sb[:, ct, osl],
                            start=(ct == 0), stop=(ct == NC_C - 1))
                    nc.vector.tensor_add(osb[:, osl], ps, bb[:, osl])
                nc.sync.dma_start(out[tt * 128:(tt + 1) * 128, :], osb)

            # ---- emission schedule (software pipeline) ----
            def emit_wp_load():
                nc.sync.dma_start(
                    wp_sb, wpT.rearrange("(n p) f -> p n f", p=128))
                nc.sync.dma_start(bp_sb, bp.ap())

            def emit_bias_bcast():
                for oc in range(C // 512):
                    osl = slice(oc * 512, (oc + 1) * 512)
                    bbp = ps_qkv.tile([128, 512], FP32, tag="ps",
                                    name=f"bbp{oc}")
                    nc.tensor.matmul(bbp, lhsT=ones_sb[0:1, :],
                                     rhs=bp_sb[:, osl], start=True, stop=True)
                    nc.vector.tensor_copy(bb[:, osl], bbp)

            # PE prewarm: ~4us of dummy matmul activity releases the HAM
            # clock-gate before the first real qkv matmul arrives
            warm = ps_sp.tile([128, 128], FP32, tag="sps", name="warm")
            for wi in range(18):
                nc.tensor.matmul(warm, lhsT=ones_sb, rhs=ones_sb,
                                 start=(wi == 0), stop=(wi == 17))

            # batch-0 qkv
            for tci in range(4):
                xc = emit_x_dma(tci)
                for ot in range(3):
                    emit_qkv_group(xc, tci, ot)

            if variant == "qkv":
                for tci in range(4, 8):
                    xc = emit_x_dma(tci)
                    for ot in range(3):
                        emit_qkv_group(xc, tci, ot)
                raise _EndEmission()

            vas = {}
            vas[(0, 0)] = emit_va(0, 0)
            vas[(0, 1)] = emit_va(0, 1)

            # filler work interleaved into batch-0 attention chunk units:
            # batch-1 qkv, deferred weight loads, batch-1 V_aug builds
            state = {"xc": None}
            # (pe_cost, fn): DMA-only items cost 0 and don't consume a slot
            fillers = []
            for tci in range(4, 8):
                fillers.append((0, lambda t=tci: state.update(
                    xc=emit_x_dma(t))))
                for ot in range(3):
                    fillers.append((1, lambda t=tci, o=ot: emit_qkv_group(
                        state["xc"], t, o)))
            fillers.append((0, emit_wp_load))
            fillers.append((1, lambda: vas.update({(1, 0): emit_va(1, 0)})))
            fillers.append((1, lambda: vas.update({(1, 1): emit_va(1, 1)})))
            fillers.append((1, emit_bias_bcast))

            def pop_fillers(n):
                budget = n
                while fillers and budget > 0:
                    cost, f = fillers.pop(0)
                    f()
                    budget -= cost

            pending = None
            for hl in range(HPC):
                for qp in range(NQC // 2):
                    pending = emit_chunk(0, hl, qp, vas[(0, hl)], pending)
                    pop_fillers(4)
            while fillers:
                fillers.pop(0)[1]()
            pending()
            pending = None
            if variant == "attn_b0":
                raise _EndEmission()
            emit_a2a(0)

            # batch-1 attention with first-half projection interleaved
            for hl in range(HPC):
                for qp in range(NQC // 2):
                    pending = emit_chunk(1, hl, qp, vas[(1, hl)], pending)
                    if hl == 0 and qp == 1:
                        emit_yt_load(0)
                    elif hl == 1 and qp == 1:
                        emit_proj(0)
                        emit_proj(1)
            pending()
            if variant == "attn":
                raise _EndEmission()
            emit_a2a(1)
            emit_yt_load(1)
            emit_proj(2)
            emit_proj(3)

      except _EndEmission:
        pass
    nc.compile()
    return nc


_NC_CACHE = None


def _get_program():
    global _NC_CACHE
    if _NC_CACHE is None:
        _NC_CACHE = build_program()
    return _NC_CACHE


# permutation of the 384 local qkv rows: tile0 = [q_h0, q_h1],
# tile1 = [k_h0, k_h1], tile2 = [v_h0, v_h1]
def _local_perm():
    p = []
    for kind in range(3):            # q, k, v
        for hl in range(HPC):
            base = hl * 3 * HD + kind * HD
            p.extend(range(base, base + HD))
    return np.array(p, dtype=np.int64)


def make_in_maps(x, w_attn, b_attn, w_proj, b_proj):
    x = np.asarray(x, dtype=np.float32)
    w_attn = np.asarray(w_attn, dtype=np.float32)
    b_attn = np.asarray(b_attn, dtype=np.float32)
    w_proj = np.asarray(w_proj, dtype=np.float32)
    b_proj = np.asarray(b_proj, dtype=np.float32)

    xT = np.ascontiguousarray(x.reshape(BT, C).T)
    wpT = np.ascontiguousarray(w_proj.T)
    bp = np.ascontiguousarray(b_proj.reshape(1, C))
    ident = np.concatenate([np.eye(HD, dtype=np.float32)] * 2, axis=0)
    ones128 = np.ones((128, 128), dtype=np.float32)
    perm = _local_perm()

    in_maps = []
    for i in range(N_CORES):
        sl = slice(i * OSL, (i + 1) * OSL)
        w_slice = w_attn[sl][perm]
        b_slice = b_attn[sl][perm]
        in_maps.append({
            "xT": xT,
            "wqT": np.ascontiguousarray(w_slice.T),
            "bq": np.ascontiguousarray(b_slice.reshape(OSL, 1)),
            "wpT": wpT,
            "bp": bp,
            "ident": ident,
            "ones128": ones128,
        })
    return in_maps


def _assemble(shards):
    """shards[j] is [TSL, C]: rows 0:TSH = b0 tokens j*TSH.., rows
    TSH:2*TSH = b1 tokens j*TSH.. Reassemble into [B, T, C]."""
    out = np.empty((BT, C), dtype=np.float32)
    for j in range(N_CORES):
        out[j * TSH:(j + 1) * TSH] = shards[j][0:TSH]
        out[T + j * TSH:T + (j + 1) * TSH] = shards[j][TSH:2 * TSH]
    return out.reshape(B, T, C)


def kernel(x, w_attn, b_attn, w_proj, b_proj):
    nc = _get_program()
    in_maps = make_in_maps(x, w_attn, b_attn, w_proj, b_proj)

    if os.environ.get("BASS_KERNEL_SIM") == "1":
        from concourse.bass_interp import MultiCoreSim
        sim = MultiCoreSim(nc, num_cores=N_CORES,
                           num_workers=int(os.environ.get("BASS_SIM_WORKERS", "8")))
        for i in range(N_CORES):
            core = sim.cores[i]
            for k, v in in_maps[i].items():
                core.tensor(k)[:] = v
        sim.simulate(check_with_hw=False)
        shards = [np.array(sim.cores[i].tensor("out")) for i in range(N_CORES)]
    else:
        last_err = None
        shards = None
        for _attempt in range(3):
            try:
                res = run_bass_kernel_spmd(nc, in_maps,
                                           core_ids=list(range(N_CORES)))
                shards = [res.results[i]["out"] for i in range(N_CORES)]
                break
            except Exception as e:  # transient device wedge: retry
                last_err = e
        if shards is None:
            raise last_err

    return _assemble(shards)


# revision 19
# speedup vs baseline: 5.1087x; 1.2303x over previous
"""Trainium2 Bass kernel for multi-head self-attention (no causal mask).

Reference computation (B=2, T=2048, C=1024, H=16, hd=64):
    qkv = x @ w_attn.T + b_attn                     # [B,T,3C], per-head interleaved
    q,k,v split per head (head h owns rows h*192 .. h*192+191 of w_attn:
        +0..63 = q, +64..127 = k, +128..191 = v)
    attn = softmax(q @ k.T * C**-0.5)               # NOTE scale uses C, no mask
    y = attn @ v -> [B,T,C] -> y @ w_proj.T + b_proj

Sharding (8 cores, tensor parallel over heads):
  - core i computes qkv^T for heads 2i,2i+1 (w_attn rows [384i, 384(i+1)))
    over all tokens, in transposed layout [384, B*T]. The per-core weight
    slice is column-permuted host-side so SBUF tile0 holds Q^T (head0 at
    partitions 0:64, head1 at 64:128), tile1 K^T, tile2 V^T — this keeps
    every matmul's lhsT/rhs at the same base partition (PE quadrant rule).
  - attention per (b, head) with scores computed directly transposed
    (S^T[kt,qt]) so the PV matmul needs no transpose of P. Softmax uses
    exp without max subtraction (scores are ~N(0, 0.25^2), max |S| < 2) and
    gets denominators for free from a ones-column appended to V.
  - emission is software-pipelined: batch-1 qkv matmuls are interleaved
    into batch-0 attention chunk units (the per-engine instruction streams
    are in-order, so emission order controls overlap), and the first half
    of the output projection is interleaved into batch-1 attention.
  - TWO AllToAll collectives (one per batch) exchange per-head outputs;
    core j ends with full-C y^T for 256 tokens of b0 plus 256 tokens of b1
    and projects them with full w_proj. The first collective is hidden
    behind batch-1 attention compute.
  - host reassembles the 8 [512, 1024] output shards (rows 0:256 = b0
    tokens j*256.., rows 256:512 = b1 tokens j*256..).

All matmuls run as float32r (TF32-like 1 cycle/row mode; plain fp32 is 4
cycles/row). fp32r requires every producer feeding a matmul to emit
fp32r-typed output (walrus verifier rule). DVE lanes cannot cross
partitions and the gpsimd partition_broadcast ucode ignores the AP base
partition on HW, so the softmax-reciprocal and bias broadcasts use K=1 PE
matmuls against a ones vector.
"""

import os
import numpy as np

import concourse.bass as bass
import concourse.bacc as bacc
import concourse.mybir as mybir
import concourse.tile as tile
from concourse.bass_utils import run_bass_kernel_spmd

N_CORES = 8
B, T, C = 2, 2048, 1024
H, HD = 16, 64
HPC = H // N_CORES          # heads per core = 2
BT = B * T                  # 4096 flattened tokens
OSL = HPC * 3 * HD          # 384 qkv rows per core
TSL = BT // N_CORES         # 512 output tokens per core
TSH = TSL // 2              # 256 tokens per batch per core
TCH = 512                   # token chunk for qkv matmul
QCH = 512                   # query chunk in attention
SCALE = float(C) ** -0.5    # softmax scale (uses C, faithful to reference)

FP32 = mybir.dt.float32
F32R = mybir.dt.float32r

NC_T = BT // TCH   # 8 token chunks
NC_C = C // 128    # 8 contraction tiles
NKT = T // 128     # 16 key tiles per batch
NQC = T // QCH     # 4 query chunks per batch


def build_program(single_core=False, variant="full"):
    nc = bacc.Bacc(
        "TRN2",
        target_bir_lowering=False,
        debug=False,
        enable_asserts=False,
        num_devices=1 if single_core else N_CORES,
    )

    xT = nc.dram_tensor("xT", [C, BT], F32R, kind="ExternalInput")
    wqT = nc.dram_tensor("wqT", [C, OSL], F32R, kind="ExternalInput")
    bq = nc.dram_tensor("bq", [OSL, 1], FP32, kind="ExternalInput")
    wpT = nc.dram_tensor("wpT", [C, C], F32R, kind="ExternalInput")
    bp = nc.dram_tensor("bp", [1, C], F32R, kind="ExternalInput")
    ident = nc.dram_tensor("ident", [128, HD], F32R, kind="ExternalInput")
    ones128 = nc.dram_tensor("ones128", [128, 128], F32R, kind="ExternalInput")
    out = nc.dram_tensor("out", [TSL, C], FP32, kind="ExternalOutput")

    class _EndEmission(Exception):
        pass

    with tile.TileContext(nc) as tc:
      try:
        with (
            tc.tile_pool(name="persist", bufs=1) as persist,
            tc.tile_pool(name="dram", bufs=1, space="DRAM") as dram,
            tc.tile_pool(name="xin", bufs=2) as xin,
            tc.tile_pool(name="ps_qkv", bufs=2, space="PSUM") as ps_qkv,
            tc.tile_pool(name="ps_sp", bufs=2, space="PSUM") as ps_sp,
            tc.tile_pool(name="ps_y", bufs=2, space="PSUM") as ps_y,
            tc.tile_pool(name="pbuf", bufs=3) as pbuf,
            tc.tile_pool(name="vbuf", bufs=3) as vbuf,
            tc.tile_pool(name="small", bufs=2) as small,
            tc.tile_pool(name="ybuf", bufs=2) as ybuf,
            tc.tile_pool(name="ytbuf", bufs=1) as ytbuf,
            tc.tile_pool(name="obuf", bufs=2) as obuf,
        ):
            # ---- persistent SBUF state (wp/bp loads deferred: keep the
            # DMA queue head free for the x chunks feeding the first qkv) ----
            id_sb = persist.tile([128, HD], F32R)
            nc.sync.dma_start(id_sb, ident.ap())
            ones_sb = persist.tile([128, 128], F32R)
            nc.sync.dma_start(ones_sb, ones128.ap())
            wq_sb = persist.tile([128, NC_C, OSL], F32R)
            _wqr = wqT.rearrange("(n p) f -> p n f", p=128)
            nc.sync.dma_start(wq_sb[:, 0:2, :], _wqr[:, 0:2, :])
            nc.sync.dma_start(wq_sb[:, 2:NC_C, :], _wqr[:, 2:NC_C, :])
            bq_sb = persist.tile([128, OSL // 128, 1], FP32)
            nc.sync.dma_start(bq_sb, bq.rearrange("(n p) o -> p n o", p=128))
            wp_sb = persist.tile([128, NC_C, C], F32R)
            bp_sb = persist.tile([1, C], F32R)

            # qkv^T, permuted layout: o-tile 0 = Q^T, 1 = K^T, 2 = V^T;
            # head0 at partitions 0:64, head1 at 64:128
            qkvT = persist.tile([128, 3, BT], F32R)
            bb = persist.tile([128, C], FP32)

            a2a_in = [dram.tile([N_CORES, 128, TSH], F32R, name=f"a2ai{b}")
                      for b in range(B)]
            a2a_out = [dram.tile([N_CORES, 128, TSH], F32R, name=f"a2ao{b}")
                       for b in range(B)]
            yT_sb = ytbuf.tile([128, NC_C, TSL], F32R)

            # ---- emission helpers ----
            def emit_x_dma(tci):
                sl = slice(tci * TCH, (tci + 1) * TCH)
                xc = xin.tile([128, NC_C, TCH], F32R, tag="xc", name=f"xc{tci}")
                xr = xT[:, sl].rearrange("(n p) f -> p n f", p=128)
                nc.sync.dma_start(xc[:, 0:2, :], xr[:, 0:2, :])
                nc.sync.dma_start(xc[:, 2:NC_C, :], xr[:, 2:NC_C, :])
                return xc

            def emit_qkv_group(xc, tci, ot):
                sl = slice(tci * TCH, (tci + 1) * TCH)
                ps = ps_qkv.tile([128, TCH], FP32, tag="ps", name=f"qk{tci}{ot}")
                for ct in range(NC_C):
                    nc.tensor.matmul(
                        ps, lhsT=wq_sb[:, ct, ot * 128:(ot + 1) * 128],
                        rhs=xc[:, ct, :], start=(ct == 0), stop=(ct == NC_C - 1))
                nc.vector.tensor_scalar_add(qkvT[:, ot, sl], ps, bq_sb[:, ot, :])

            def emit_va(b, hl):
                hp = hl * HD
                boff = b * T
                v_ap = qkvT[hp:hp + HD, 2, boff:boff + T]
                va = vbuf.tile([128, NKT, HD + 1], F32R, tag="va",
                               name=f"va{b}{hl}")
                nc.sync.dma_start(va[:, :, HD], ones128.ap()[:, 0:NKT])
                for k in range(NKT):
                    tp = ps_qkv.tile([128, HD], F32R, tag="ps", name=f"tp{k}")
                    nc.tensor.transpose(
                        tp, in_=v_ap[:, k * 128:(k + 1) * 128],
                        identity=id_sb[hp:hp + HD, :])
                    nc.vector.tensor_copy(va[:, k, 0:HD], tp)
                return va

            def emit_chunk(b, hl, qp, va, prev_finish=None):
                """One attention pair unit: S^T/exp/PV over all kt for TWO
                512-token query chunks (one 1024-wide exp per kt tile).
                Normalization+staging is returned as a closure deferred into
                the next unit (keeps the PE stream from stalling on the DVE
                reciprocal at unit boundaries)."""
                hp = hl * HD
                boff = b * T
                q_ap = qkvT[hp:hp + HD, 0, boff:boff + T]
                k_ap = qkvT[hp:hp + HD, 1, boff:boff + T]
                W = 2 * QCH
                qsl = slice(qp * W, (qp + 1) * W)
                ypx = [ps_y.tile([128, QCH], FP32, tag="ypx",
                                 name=f"y{b}{hl}{qp}{h}") for h in range(2)]
                for k in range(NKT):
                    sps = ps_sp.tile([128, W], FP32, tag="sps", name=f"s{k}")
                    for h in range(2):
                        nc.tensor.matmul(
                            sps[:, h * QCH:(h + 1) * QCH],
                            lhsT=k_ap[:, k * 128:(k + 1) * 128],
                            rhs=q_ap[:, qp * W + h * QCH:
                                     qp * W + (h + 1) * QCH],
                            start=True, stop=True)
                    if k == 0 and prev_finish is not None:
                        prev_finish()
                        prev_finish = None
                    pt = pbuf.tile([128, W], F32R, tag="pt", name=f"p{k}")
                    nc.scalar.activation(
                        pt, sps, mybir.ActivationFunctionType.Exp, scale=SCALE)
                    for h in range(2):
                        nc.tensor.matmul(
                            ypx[h][0:HD + 1, :], lhsT=va[:, k, :],
                            rhs=pt[:, h * QCH:(h + 1) * QCH],
                            start=(k == 0), stop=(k == NKT - 1))
                if prev_finish is not None:
                    prev_finish()

                def finish():
                    # normalize by the denominators accumulated in
                    # partition HD of each accumulator
                    for h in range(2):
                        qc = 2 * qp + h
                        rec = small.tile([HD + 1, QCH], F32R, tag="rec",
                                         name="rec")
                        with nc.allow_low_precision(reason="f32r recip"):
                            nc.vector.reciprocal(rec[HD:HD + 1, :],
                                                 ypx[h][HD:HD + 1, :])
                        rbp = ps_sp.tile([HD, QCH], FP32, tag="sps",
                                         name="rbp")
                        nc.tensor.matmul(rbp, lhsT=ones_sb[HD:HD + 1, 0:HD],
                                         rhs=rec[HD:HD + 1, :],
                                         start=True, stop=True)
                        rb = small.tile([HD, QCH], FP32, tag="rb", name="rb")
                        nc.vector.tensor_copy(rb, rbp)
                        yc = ybuf.tile([HD, QCH], F32R, tag="yc", name="yc")
                        nc.vector.tensor_tensor(yc, ypx[h][0:HD, :], rb,
                                                op=mybir.AluOpType.mult)
                        # dest core j holds batch tokens [j*TSH, (j+1)*TSH)
                        for half in range(2):
                            j = 2 * qc + half
                            nc.sync.dma_start(
                                a2a_in[b][j, hp:hp + HD, :],
                                yc[:, half * TSH:(half + 1) * TSH])
                return finish

            def emit_a2a(b):
                if single_core or variant == "noccl":
                    nc.sync.dma_start(a2a_out[b].opt(), a2a_in[b].opt())
                else:
                    nc.gpsimd.collective_compute(
                        "AllToAll", mybir.AluOpType.bypass,
                        replica_groups=[list(range(N_CORES))],
                        ins=[a2a_in[b].opt()], outs=[a2a_out[b].opt()])

            def emit_yt_load(b):
                for ct in range(NC_C):
                    nc.sync.dma_start(
                        yT_sb[:, ct, b * TSH:(b + 1) * TSH], a2a_out[b][ct])

            def emit_proj(tt):
                osb = obuf.tile([128, C], FP32, tag="osb", name=f"o{tt}")
                for oc in range(C // 512):
                    osl = slice(oc * 512, (oc + 1) * 512)
                    ps = ps_qkv.tile([128, 512], FP32, tag="ps", name=f"op{tt}{oc}")
                    for ct in range(NC_C):
                        nc.tensor.matmul(
                            ps, lhsT=yT_sb[:, ct, tt * 128:(tt + 1) * 128],
                            rhs=wp_sb[:, ct, osl],
                            start=(ct == 0), stop=(ct == NC_C - 1))
                    nc.vector.tensor_add(osb[:, osl], ps, bb[:, osl])
                nc.sync.dma_start(out[tt * 128:(tt + 1) * 128, :], osb)

            # ---- emission schedule (software pipeline) ----
            def emit_wp_load():
                nc.sync.dma_start(
                    wp_sb, wpT.rearrange("(n p) f -> p n f", p=128))
                nc.sync.dma_start(bp_sb, bp.ap())

            def emit_bias_bcast():
                for oc in range(C // 512):
                    osl = slice(oc * 512, (oc + 1) * 512)
                    bbp = ps_qkv.tile([128, 512], FP32, tag="ps",
                                    name=f"bbp{oc}")
                    nc.tensor.matmul(bbp, lhsT=ones_sb[0:1, :],
                                     rhs=bp_sb[:, osl], start=True, stop=True)
                    nc.vector.tensor_copy(bb[:, osl], bbp)

            # PE prewarm: ~4us of dummy matmul activity releases the HAM
            # clock-gate before the first real qkv matmul arrives
            warm = ps_sp.tile([128, 128], FP32, tag="sps", name="warm")
            for wi in range(18):
                nc.tensor.matmul(warm, lhsT=ones_sb, rhs=ones_sb,
                                 start=(wi == 0), stop=(wi == 17))

            # batch-0 qkv
            for tci in range(4):
                xc = emit_x_dma(tci)
                for ot in range(3):
                    emit_qkv_group(xc, tci, ot)

            if variant == "qkv":
                for tci in range(4, 8):
                    xc = emit_x_dma(tci)
                    for ot in range(3):
                        emit_qkv_group(xc, tci, ot)
                raise _EndEmission()

            vas = {}
            vas[(0, 0)] = emit_va(0, 0)
            vas[(0, 1)] = emit_va(0, 1)

            # filler work interleaved into batch-0 attention chunk units:
            # batch-1 qkv, deferred weight loads, batch-1 V_aug builds
            state = {"xc": None}
            # (pe_cost, fn): DMA-only items cost 0 and don't consume a slot
            fillers = []
            for tci in range(4, 8):
                fillers.append((0, lambda t=tci: state.update(
                    xc=emit_x_dma(t))))
                for ot in range(3):
                    fillers.append((1, lambda t=tci, o=ot: emit_qkv_group(
                        state["xc"], t, o)))
            fillers.append((0, emit_wp_load))
            fillers.append((1, lambda: vas.update({(1, 0): emit_va(1, 0)})))
            fillers.append((1, lambda: vas.update({(1, 1): emit_va(1, 1)})))
            fillers.append((1, emit_bias_bcast))

            def pop_fillers(n):
                budget = n
                while fillers and budget > 0:
                    cost, f = fillers.pop(0)
                    f()
                    budget -= cost

            pending = None
            for hl in range(HPC):
                for qp in range(NQC // 2):
                    pending = emit_chunk(0, hl, qp, vas[(0, hl)], pending)
                    pop_fillers(4)
            while fillers:
                fillers.pop(0)[1]()
            pending()
            pending = None
            if variant == "attn_b0":
                raise _EndEmission()
            emit_a2a(0)

            # batch-1 attention with first-half projection interleaved
            for hl in range(HPC):
                for qp in range(NQC // 2):
                    pending = emit_chunk(1, hl, qp, vas[(1, hl)], pending)
                    if hl == 0 and qp == 1:
                        emit_yt_load(0)
                    elif hl == 1 and qp == 1:
                        emit_proj(0)
                        emit_proj(1)
            pending()
            if variant == "attn":
                raise _EndEmission()
            emit_a2a(1)
            emit_yt_load(1)
            emit_proj(2)
            emit_proj(3)

      except _EndEmission:
        pass
    nc.compile()
    return nc


_NC_CACHE = None


def _get_program():
    global _NC_CACHE
    if _NC_CACHE is None:
        _NC_CACHE = build_program()
    return _NC_CACHE


# permutation of the 384 local qkv rows: tile0 = [q_h0, q_h1],
# tile1 = [k_h0, k_h1], tile2 = [v_h0, v_h1]
def _local_perm():
    p = []
    for kind in range(3):            # q, k, v
        for hl in range(HPC):
            base = hl * 3 * HD + kind * HD
            p.extend(range(base, base + HD))
    return np.array(p, dtype=np.int64)


def make_in_maps(x, w_attn, b_attn, w_proj, b_proj):
    x = np.asarray(x, dtype=np.float32)
    w_attn = np.asarray(w_attn, dtype=np.float32)
    b_attn = np.asarray(b_attn, dtype=np.float32)
    w_proj = np.asarray(w_proj, dtype=np.float32)
    b_proj = np.asarray(b_proj, dtype=np.float32)

    xT = np.ascontiguousarray(x.reshape(BT, C).T)
    wpT = np.ascontiguousarray(w_proj.T)
    bp = np.ascontiguousarray(b_proj.reshape(1, C))
    ident = np.concatenate([np.eye(HD, dtype=np.float32)] * 2, axis=0)
    ones128 = np.ones((128, 128), dtype=np.float32)
    perm = _local_perm()

    in_maps = []
    for i in range(N_CORES):
        sl = slice(i * OSL, (i + 1) * OSL)
        w_slice = w_attn[sl][perm]
        b_slice = b_attn[sl][perm]
        in_maps.append({
            "xT": xT,
            "wqT": np.ascontiguousarray(w_slice.T),
            "bq": np.ascontiguousarray(b_slice.reshape(OSL, 1)),
            "wpT": wpT,
            "bp": bp,
            "ident": ident,
            "ones128": ones128,
        })
    return in_maps


def _assemble(shards):
    """shards[j] is [TSL, C]: rows 0:TSH = b0 tokens j*TSH.., rows
    TSH:2*TSH = b1 tokens j*TSH.. Reassemble into [B, T, C]."""
    out = np.empty((BT, C), dtype=np.float32)
    for j in range(N_CORES):
        out[j * TSH:(j + 1) * TSH] = shards[j][0:TSH]
        out[T + j * TSH:T + (j + 1) * TSH] = shards[j][TSH:2 * TSH]
    return out.reshape(B, T, C)


def kernel(x, w_attn, b_attn, w_proj, b_proj):
    nc = _get_program()
    in_maps = make_in_maps(x, w_attn, b_attn, w_proj, b_proj)

    if os.environ.get("BASS_KERNEL_SIM") == "1":
        from concourse.bass_interp import MultiCoreSim
        sim = MultiCoreSim(nc, num_cores=N_CORES,
                           num_workers=int(os.environ.get("BASS_SIM_WORKERS", "8")))
        for i in range(N_CORES):
            core = sim.cores[i]
            for k, v in in_maps[i].items():
                core.tensor(k)[:] = v
        sim.simulate(check_with_hw=False)
        shards = [np.array(sim.cores[i].tensor("out")) for i in range(N_CORES)]
    else:
        last_err = None
        shards = None
        for _attempt in range(3):
            try:
                res = run_bass_kernel_spmd(nc, in_maps,
                                           core_ids=list(range(N_CORES)))
                shards = [res.results[i]["out"] for i in range(N_CORES)]
                break
            except Exception as e:  # transient device wedge: retry
                last_err = e
        if shards is None:
            raise last_err

    return _assemble(shards)


# revision 21
# speedup vs baseline: 5.4011x; 1.0572x over previous
"""Trainium2 Bass kernel for multi-head self-attention (no causal mask).

Reference computation (B=2, T=2048, C=1024, H=16, hd=64):
    qkv = x @ w_attn.T + b_attn                     # [B,T,3C], per-head interleaved
    q,k,v split per head (head h owns rows h*192 .. h*192+191 of w_attn:
        +0..63 = q, +64..127 = k, +128..191 = v)
    attn = softmax(q @ k.T * C**-0.5)               # NOTE scale uses C, no mask
    y = attn @ v -> [B,T,C] -> y @ w_proj.T + b_proj

Sharding (8 cores, tensor parallel over heads):
  - core i computes qkv^T for heads 2i,2i+1 (w_attn rows [384i, 384(i+1)))
    over all tokens, in transposed layout [384, B*T]. The per-core weight
    slice is column-permuted host-side so SBUF tile0 holds Q^T (head0 at
    partitions 0:64, head1 at 64:128), tile1 K^T, tile2 V^T — this keeps
    every matmul's lhsT/rhs at the same base partition (PE quadrant rule).
  - attention per (b, head) with scores computed directly transposed
    (S^T[kt,qt]) so the PV matmul needs no transpose of P. Softmax uses
    exp without max subtraction (scores are ~N(0, 0.25^2), max |S| < 2) and
    gets denominators for free from a ones-column appended to V.
  - emission is software-pipelined: batch-1 qkv matmuls are interleaved
    into batch-0 attention chunk units (the per-engine instruction streams
    are in-order, so emission order controls overlap), and the first half
    of the output projection is interleaved into batch-1 attention.
  - TWO AllToAll collectives (one per batch) exchange per-head outputs;
    core j ends with full-C y^T for 256 tokens of b0 plus 256 tokens of b1
    and projects them with full w_proj. The first collective is hidden
    behind batch-1 attention compute.
  - host reassembles the 8 [512, 1024] output shards (rows 0:256 = b0
    tokens j*256.., rows 256:512 = b1 tokens j*256..).

All matmuls run as float32r (TF32-like 1 cycle/row mode; plain fp32 is 4
cycles/row). fp32r requires every producer feeding a matmul to emit
fp32r-typed output (walrus verifier rule). DVE lanes cannot cross
partitions and the gpsimd partition_broadcast ucode ignores the AP base
partition on HW, so the softmax-reciprocal and bias broadcasts use K=1 PE
matmuls against a ones vector.
"""

import os
import numpy as np

import concourse.bass as bass
import concourse.bacc as bacc
import concourse.mybir as mybir
import concourse.tile as tile
from concourse.bass_utils import run_bass_kernel_spmd

N_CORES = 8
B, T, C = 2, 2048, 1024
H, HD = 16, 64
HPC = H // N_CORES          # heads per core = 2
BT = B * T                  # 4096 flattened tokens
OSL = HPC * 3 * HD          # 384 qkv rows per core
TSL = BT // N_CORES         # 512 output tokens per core
TSH = TSL // 2              # 256 tokens per batch per core
TCH = 512                   # token chunk for qkv matmul
QCH = 512                   # query chunk in attention
SCALE = float(C) ** -0.5    # softmax scale (uses C, faithful to reference)

FP32 = mybir.dt.float32
F32R = mybir.dt.float32r

NC_T = BT // TCH   # 8 token chunks
NC_C = C // 128    # 8 contraction tiles
NKT = T // 128     # 16 key tiles per batch
NQC = T // QCH     # 4 query chunks per batch


def build_program(single_core=False, variant="full"):
    nc = bacc.Bacc(
        "TRN2",
        target_bir_lowering=False,
        debug=False,
        enable_asserts=False,
        num_devices=1 if single_core else N_CORES,
    )

    xT = nc.dram_tensor("xT", [C, BT], F32R, kind="ExternalInput")
    wqT = nc.dram_tensor("wqT", [C, OSL], F32R, kind="ExternalInput")
    bq = nc.dram_tensor("bq", [OSL, 1], FP32, kind="ExternalInput")
    wpT = nc.dram_tensor("wpT", [C, C], F32R, kind="ExternalInput")
    bp = nc.dram_tensor("bp", [1, C], F32R, kind="ExternalInput")
    ident = nc.dram_tensor("ident", [128, HD], F32R, kind="ExternalInput")
    ones128 = nc.dram_tensor("ones128", [128, 128], F32R, kind="ExternalInput")
    out = nc.dram_tensor("out", [TSL, C], FP32, kind="ExternalOutput")

    class _EndEmission(Exception):
        pass

    with tile.TileContext(nc) as tc:
      try:
        with (
            tc.tile_pool(name="persist", bufs=1) as persist,
            tc.tile_pool(name="dram", bufs=1, space="DRAM") as dram,
            tc.tile_pool(name="xin", bufs=2) as xin,
            tc.tile_pool(name="ps_qkv", bufs=2, space="PSUM") as ps_qkv,
            tc.tile_pool(name="ps_sp", bufs=2, space="PSUM") as ps_sp,
            tc.tile_pool(name="ps_y", bufs=2, space="PSUM") as ps_y,
            tc.tile_pool(name="pbuf", bufs=4) as pbuf,
            tc.tile_pool(name="vbuf", bufs=3) as vbuf,
            tc.tile_pool(name="small", bufs=2) as small,
            tc.tile_pool(name="ybuf", bufs=2) as ybuf,
            tc.tile_pool(name="ytbuf", bufs=1) as ytbuf,
            tc.tile_pool(name="obuf", bufs=2) as obuf,
        ):
            # ---- persistent SBUF state (wp/bp loads deferred: keep the
            # DMA queue head free for the x chunks feeding the first qkv) ----
            id_sb = persist.tile([128, HD], F32R)
            nc.sync.dma_start(id_sb, ident.ap())
            ones_sb = persist.tile([128, 128], F32R)
            nc.sync.dma_start(ones_sb, ones128.ap())
            wq_sb = persist.tile([128, NC_C, OSL], F32R)
            _wqr = wqT.rearrange("(n p) f -> p n f", p=128)
            nc.sync.dma_start(wq_sb[:, 0:2, :], _wqr[:, 0:2, :])
            nc.sync.dma_start(wq_sb[:, 2:NC_C, :], _wqr[:, 2:NC_C, :])
            bq_sb = persist.tile([128, OSL // 128, 1], FP32)
            nc.sync.dma_start(bq_sb, bq.rearrange("(n p) o -> p n o", p=128))
            wp_sb = persist.tile([128, NC_C, C], F32R)
            bp_sb = persist.tile([1, C], F32R)

            # qkv^T, permuted layout: o-tile 0 = Q^T, 1 = K^T, 2 = V^T;
            # head0 at partitions 0:64, head1 at 64:128
            qkvT = persist.tile([128, 3, BT], F32R)
            bb = persist.tile([128, C], FP32)

            a2a_in = [dram.tile([N_CORES, 128, TSH], F32R, name=f"a2ai{b}")
                      for b in range(B)]
            a2a_out = [dram.tile([N_CORES, 128, TSH], F32R, name=f"a2ao{b}")
                       for b in range(B)]
            yT_sb = ytbuf.tile([128, NC_C, TSL], F32R)

            # ---- emission helpers ----
            def emit_x_dma(tci):
                sl = slice(tci * TCH, (tci + 1) * TCH)
                xc = xin.tile([128, NC_C, TCH], F32R, tag="xc", name=f"xc{tci}")
                xr = xT[:, sl].rearrange("(n p) f -> p n f", p=128)
                nc.sync.dma_start(xc[:, 0:2, :], xr[:, 0:2, :])
                nc.sync.dma_start(xc[:, 2:NC_C, :], xr[:, 2:NC_C, :])
                return xc

            def emit_qkv_group(xc, tci, ot):
                sl = slice(tci * TCH, (tci + 1) * TCH)
                ps = ps_qkv.tile([128, TCH], FP32, tag="ps", name=f"qk{tci}{ot}")
                for ct in range(NC_C):
                    nc.tensor.matmul(
                        ps, lhsT=wq_sb[:, ct, ot * 128:(ot + 1) * 128],
                        rhs=xc[:, ct, :], start=(ct == 0), stop=(ct == NC_C - 1))
                nc.vector.tensor_scalar_add(qkvT[:, ot, sl], ps, bq_sb[:, ot, :])

            def emit_va(b, hl):
                hp = hl * HD
                boff = b * T
                v_ap = qkvT[hp:hp + HD, 2, boff:boff + T]
                va = vbuf.tile([128, NKT, HD + 1], F32R, tag="va",
                               name=f"va{b}{hl}")
                nc.sync.dma_start(va[:, :, HD], ones128.ap()[:, 0:NKT])
                for k in range(NKT):
                    tp = ps_qkv.tile([128, HD], F32R, tag="ps", name=f"tp{k}")
                    nc.tensor.transpose(
                        tp, in_=v_ap[:, k * 128:(k + 1) * 128],
                        identity=id_sb[hp:hp + HD, :])
                    nc.vector.tensor_copy(va[:, k, 0:HD], tp)
                return va

            def emit_chunk(b, hl, qp, va, prev_finish=None):
                """One attention pair unit: S^T/exp/PV over all kt for TWO
                512-token query chunks (one 1024-wide exp per kt tile).
                Normalization+staging is returned as a closure deferred into
                the next unit (keeps the PE stream from stalling on the DVE
                reciprocal at unit boundaries)."""
                hp = hl * HD
                boff = b * T
                q_ap = qkvT[hp:hp + HD, 0, boff:boff + T]
                k_ap = qkvT[hp:hp + HD, 1, boff:boff + T]
                W = 2 * QCH
                qsl = slice(qp * W, (qp + 1) * W)
                ypx = [ps_y.tile([128, QCH], FP32, tag="ypx",
                                 name=f"y{b}{hl}{qp}{h}") for h in range(2)]
                for k in range(NKT):
                    sps = ps_sp.tile([128, W], FP32, tag="sps", name=f"s{k}")
                    for h in range(2):
                        nc.tensor.matmul(
                            sps[:, h * QCH:(h + 1) * QCH],
                            lhsT=k_ap[:, k * 128:(k + 1) * 128],
                            rhs=q_ap[:, qp * W + h * QCH:
                                     qp * W + (h + 1) * QCH],
                            start=True, stop=True)
                    if k == 0 and prev_finish is not None:
                        prev_finish()
                        prev_finish = None
                    pt = pbuf.tile([128, W], F32R, tag="pt", name=f"p{k}")
                    nc.scalar.activation(
                        pt, sps, mybir.ActivationFunctionType.Exp, scale=SCALE)
                    for h in range(2):
                        nc.tensor.matmul(
                            ypx[h][0:HD + 1, :], lhsT=va[:, k, :],
                            rhs=pt[:, h * QCH:(h + 1) * QCH],
                            start=(k == 0), stop=(k == NKT - 1))
                if prev_finish is not None:
                    prev_finish()

                def finish():
                    # normalize by the denominators accumulated in
                    # partition HD of each accumulator
                    for h in range(2):
                        qc = 2 * qp + h
                        rec = small.tile([HD + 1, QCH], F32R, tag="rec",
                                         name="rec")
                        with nc.allow_low_precision(reason="f32r recip"):
                            nc.vector.reciprocal(rec[HD:HD + 1, :],
                                                 ypx[h][HD:HD + 1, :])
                        rbp = ps_sp.tile([HD, QCH], FP32, tag="sps",
                                         name="rbp")
                        nc.tensor.matmul(rbp, lhsT=ones_sb[HD:HD + 1, 0:HD],
                                         rhs=rec[HD:HD + 1, :],
                                         start=True, stop=True)
                        rb = small.tile([HD, QCH], FP32, tag="rb", name="rb")
                        nc.vector.tensor_copy(rb, rbp)
                        yc = ybuf.tile([HD, QCH], F32R, tag="yc", name="yc")
                        nc.vector.tensor_tensor(yc, ypx[h][0:HD, :], rb,
                                                op=mybir.AluOpType.mult)
                        # dest core j holds batch tokens [j*TSH, (j+1)*TSH)
                        for half in range(2):
                            j = 2 * qc + half
                            nc.sync.dma_start(
                                a2a_in[b][j, hp:hp + HD, :],
                                yc[:, half * TSH:(half + 1) * TSH])
                return finish

            def emit_a2a(b):
                if single_core or variant == "noccl":
                    nc.sync.dma_start(a2a_out[b].opt(), a2a_in[b].opt())
                else:
                    nc.gpsimd.collective_compute(
                        "AllToAll", mybir.AluOpType.bypass,
                        replica_groups=[list(range(N_CORES))],
                        ins=[a2a_in[b].opt()], outs=[a2a_out[b].opt()])

            def emit_yt_load(b):
                for ct in range(NC_C):
                    nc.sync.dma_start(
                        yT_sb[:, ct, b * TSH:(b + 1) * TSH], a2a_out[b][ct])

            def emit_proj(tt):
                osb = obuf.tile([128, C], FP32, tag="osb", name=f"o{tt}")
                for oc in range(C // 512):
                    osl = slice(oc * 512, (oc + 1) * 512)
                    ps = ps_qkv.tile([128, 512], FP32, tag="ps", name=f"op{tt}{oc}")
                    for ct in range(NC_C):
                        nc.tensor.matmul(
                            ps, lhsT=yT_sb[:, ct, tt * 128:(tt + 1) * 128],
                            rhs=wp_sb[:, ct, osl],
                            start=(ct == 0), stop=(ct == NC_C - 1))
                    nc.vector.tensor_add(osb[:, osl], ps, bb[:, osl])
                nc.sync.dma_start(out[tt * 128:(tt + 1) * 128, :], osb)

            # ---- emission schedule (software pipeline) ----
            def emit_wp_load():
                nc.sync.dma_start(
                    wp_sb, wpT.rearrange("(n p) f -> p n f", p=128))
                nc.sync.dma_start(bp_sb, bp.ap())

            def emit_bias_bcast():
                for oc in range(C // 512):
                    osl = slice(oc * 512, (oc + 1) * 512)
                    bbp = ps_qkv.tile([128, 512], FP32, tag="ps",
                                    name=f"bbp{oc}")
                    nc.tensor.matmul(bbp, lhsT=ones_sb[0:1, :],
                                     rhs=bp_sb[:, osl], start=True, stop=True)
                    nc.vector.tensor_copy(bb[:, osl], bbp)

            # PE prewarm: ~4us of dummy matmul activity releases the HAM
            # clock-gate before the first real qkv matmul arrives
            warm = ps_sp.tile([128, 128], FP32, tag="sps", name="warm")
            for wi in range(18):
                nc.tensor.matmul(warm, lhsT=ones_sb, rhs=ones_sb,
                                 start=(wi == 0), stop=(wi == 17))

            # batch-0 qkv
            for tci in range(4):
                xc = emit_x_dma(tci)
                for ot in range(3):
                    emit_qkv_group(xc, tci, ot)

            if variant == "qkv":
                for tci in range(4, 8):
                    xc = emit_x_dma(tci)
                    for ot in range(3):
                        emit_qkv_group(xc, tci, ot)
                raise _EndEmission()

            vas = {}
            vas[(0, 0)] = emit_va(0, 0)
            vas[(0, 1)] = emit_va(0, 1)

            # filler work interleaved into batch-0 attention chunk units:
            # batch-1 qkv, deferred weight loads, batch-1 V_aug builds
            state = {"xc": None}
            # (pe_cost, fn): DMA-only items cost 0 and don't consume a slot
            fillers = []
            for tci in range(4, 8):
                fillers.append((0, lambda t=tci: state.update(
                    xc=emit_x_dma(t))))
                for ot in range(3):
                    fillers.append((1, lambda t=tci, o=ot: emit_qkv_group(
                        state["xc"], t, o)))
            fillers.append((0, emit_wp_load))
            fillers.append((1, lambda: vas.update({(1, 0): emit_va(1, 0)})))
            fillers.append((1, lambda: vas.update({(1, 1): emit_va(1, 1)})))
            fillers.append((1, emit_bias_bcast))

            def pop_fillers(n):
                budget = n
                while fillers and budget > 0:
                    cost, f = fillers.pop(0)
                    f()
                    budget -= cost

            pending = None
            for hl in range(HPC):
                for qp in range(NQC // 2):
                    pending = emit_chunk(0, hl, qp, vas[(0, hl)], pending)
                    pop_fillers(4)
            while fillers:
                fillers.pop(0)[1]()
            pending()
            pending = None
            if variant == "attn_b0":
                raise _EndEmission()
            emit_a2a(0)

            # batch-1 attention with first-half projection interleaved
            for hl in range(HPC):
                for qp in range(NQC // 2):
                    pending = emit_chunk(1, hl, qp, vas[(1, hl)], pending)
                    if hl == 0 and qp == 1:
                        emit_yt_load(0)
                    elif hl == 1 and qp == 1:
                        emit_proj(0)
                        emit_proj(1)
            pending()
            if variant == "attn":
                raise _EndEmission()
            emit_a2a(1)
            emit_yt_load(1)
            emit_proj(2)
            emit_proj(3)

      except _EndEmission:
        pass
    nc.compile()
    return nc


_NC_CACHE = None


def _get_program():
    global _NC_CACHE
    if _NC_CACHE is None:
        _NC_CACHE = build_program()
    return _NC_CACHE


# permutation of the 384 local qkv rows: tile0 = [q_h0, q_h1],
# tile1 = [k_h0, k_h1], tile2 = [v_h0, v_h1]
def _local_perm():
    p = []
    for kind in range(3):            # q, k, v
        for hl in range(HPC):
            base = hl * 3 * HD + kind * HD
            p.extend(range(base, base + HD))
    return np.array(p, dtype=np.int64)


def make_in_maps(x, w_attn, b_attn, w_proj, b_proj):
    x = np.asarray(x, dtype=np.float32)
    w_attn = np.asarray(w_attn, dtype=np.float32)
    b_attn = np.asarray(b_attn, dtype=np.float32)
    w_proj = np.asarray(w_proj, dtype=np.float32)
    b_proj = np.asarray(b_proj, dtype=np.float32)

    xT = np.ascontiguousarray(x.reshape(BT, C).T)
    wpT = np.ascontiguousarray(w_proj.T)
    bp = np.ascontiguousarray(b_proj.reshape(1, C))
    ident = np.concatenate([np.eye(HD, dtype=np.float32)] * 2, axis=0)
    ones128 = np.ones((128, 128), dtype=np.float32)
    perm = _local_perm()

    in_maps = []
    for i in range(N_CORES):
        sl = slice(i * OSL, (i + 1) * OSL)
        w_slice = w_attn[sl][perm]
        b_slice = b_attn[sl][perm]
        in_maps.append({
            "xT": xT,
            "wqT": np.ascontiguousarray(w_slice.T),
            "bq": np.ascontiguousarray(b_slice.reshape(OSL, 1)),
            "wpT": wpT,
            "bp": bp,
            "ident": ident,
            "ones128": ones128,
        })
    return in_maps


def _assemble(shards):
    """shards[j] is [TSL, C]: rows 0:TSH = b0 tokens j*TSH.., rows
    TSH:2*TSH = b1 tokens j*TSH.. Reassemble into [B, T, C]."""
    out = np.empty((BT, C), dtype=np.float32)
    for j in range(N_CORES):
        out[j * TSH:(j + 1) * TSH] = shards[j][0:TSH]
        out[T + j * TSH:T + (j + 1) * TSH] = shards[j][TSH:2 * TSH]
    return out.reshape(B, T, C)


def kernel(x, w_attn, b_attn, w_proj, b_proj):
    nc = _get_program()
    in_maps = make_in_maps(x, w_attn, b_attn, w_proj, b_proj)

    if os.environ.get("BASS_KERNEL_SIM") == "1":
        from concourse.bass_interp import MultiCoreSim
        sim = MultiCoreSim(nc, num_cores=N_CORES,
                           num_workers=int(os.environ.get("BASS_SIM_WORKERS", "8")))
        for i in range(N_CORES):
            core = sim.cores[i]
            for k, v in in_maps[i].items():
                core.tensor(k)[:] = v
        sim.simulate(check_with_hw=False)
        shards = [np.array(sim.cores[i].tensor("out")) for i in range(N_CORES)]
    else:
        last_err = None
        shards = None
        for _attempt in range(3):
            try:
                res = run_bass_kernel_spmd(nc, in_maps,
                                           core_ids=list(range(N_CORES)))
                shards = [res.results[i]["out"] for i in range(N_CORES)]
                break
            except Exception as e:  # transient device wedge: retry
                last_err = e
        if shards is None:
            raise last_err

    return _assemble(shards)


# revision 23
# speedup vs baseline: 9.3860x; 1.7378x over previous
"""Trainium2 Bass kernel for multi-head self-attention (no causal mask).

Reference computation (B=2, T=2048, C=1024, H=16, hd=64):
    qkv = x @ w_attn.T + b_attn                     # [B,T,3C], per-head interleaved
    q,k,v split per head (head h owns rows h*192 .. h*192+191 of w_attn:
        +0..63 = q, +64..127 = k, +128..191 = v)
    attn = softmax(q @ k.T * C**-0.5)               # NOTE scale uses C, no mask
    y = attn @ v -> [B,T,C] -> y @ w_proj.T + b_proj

Sharding (8 cores, tensor parallel over heads):
  - core i computes qkv^T for heads 2i,2i+1 (w_attn rows [384i, 384(i+1)))
    over all tokens, in transposed layout [384, B*T]. The per-core weight
    slice is column-permuted host-side so SBUF tile0 holds Q^T (head0 at
    partitions 0:64, head1 at 64:128), tile1 K^T, tile2 V^T — this keeps
    every matmul's lhsT/rhs at the same base partition (PE quadrant rule).
  - attention per (b, head) with scores computed directly transposed
    (S^T[kt,qt]) so the PV matmul needs no transpose of P. Softmax uses
    exp without max subtraction (scores are ~N(0, 0.25^2), max |S| < 2) and
    gets denominators for free from a ones-column appended to V.
  - emission is software-pipelined: batch-1 qkv matmuls are interleaved
    into batch-0 attention chunk units (the per-engine instruction streams
    are in-order, so emission order controls overlap), and the first half
    of the output projection is interleaved into batch-1 attention.
  - TWO AllToAll collectives (one per batch) exchange per-head outputs;
    core j ends with full-C y^T for 256 tokens of b0 plus 256 tokens of b1
    and projects them with full w_proj. The first collective is hidden
    behind batch-1 attention compute.
  - host reassembles the 8 [512, 1024] output shards (rows 0:256 = b0
    tokens j*256.., rows 256:512 = b1 tokens j*256..).

All matmuls run as float32r (TF32-like 1 cycle/row mode; plain fp32 is 4
cycles/row). fp32r requires every producer feeding a matmul to emit
fp32r-typed output (walrus verifier rule). DVE lanes cannot cross
partitions and the gpsimd partition_broadcast ucode ignores the AP base
partition on HW, so the softmax-reciprocal and bias broadcasts use K=1 PE
matmuls against a ones vector.
"""

import os
import numpy as np

import concourse.bass as bass
import concourse.bacc as bacc
import concourse.mybir as mybir
import concourse.tile as tile
from concourse.bass_utils import run_bass_kernel_spmd

N_CORES = 8
B, T, C = 2, 2048, 1024
H, HD = 16, 64
HPC = H // N_CORES          # heads per core = 2
BT = B * T                  # 4096 flattened tokens
OSL = HPC * 3 * HD          # 384 qkv rows per core
TSL = BT // N_CORES         # 512 output tokens per core
TSH = TSL // 2              # 256 tokens per batch per core
TCH = 512                   # token chunk for qkv matmul
QCH = 512                   # query chunk in attention
SCALE = float(C) ** -0.5    # softmax scale (uses C, faithful to reference)

FP32 = mybir.dt.float32
F32R = mybir.dt.float32r

NC_T = BT // TCH   # 8 token chunks
NC_C = C // 128    # 8 contraction tiles
NKT = T // 128     # 16 key tiles per batch
NQC = T // QCH     # 4 query chunks per batch


def build_program(single_core=False, variant="full"):
    nc = bacc.Bacc(
        "TRN2",
        target_bir_lowering=False,
        debug=False,
        enable_asserts=False,
        num_devices=1 if single_core else N_CORES,
    )

    xT = nc.dram_tensor("xT", [C, BT], F32R, kind="ExternalInput")
    wqT = nc.dram_tensor("wqT", [C, OSL], F32R, kind="ExternalInput")
    bq = nc.dram_tensor("bq", [OSL, 1], FP32, kind="ExternalInput")
    wpT = nc.dram_tensor("wpT", [C, C], F32R, kind="ExternalInput")
    bp = nc.dram_tensor("bp", [1, C], F32R, kind="ExternalInput")
    ident = nc.dram_tensor("ident", [128, HD], F32R, kind="ExternalInput")
    ones128 = nc.dram_tensor("ones128", [128, 128], F32R, kind="ExternalInput")
    out = nc.dram_tensor("out", [TSL, C], FP32, kind="ExternalOutput")

    class _EndEmission(Exception):
        pass

    with tile.TileContext(nc) as tc:
      try:
        with (
            tc.tile_pool(name="persist", bufs=1) as persist,
            tc.tile_pool(name="dram", bufs=1, space="DRAM") as dram,
            tc.tile_pool(name="xin", bufs=2) as xin,
            tc.tile_pool(name="ps_qkv", bufs=2, space="PSUM") as ps_qkv,
            tc.tile_pool(name="ps_sp", bufs=2, space="PSUM") as ps_sp,
            tc.tile_pool(name="ps_y", bufs=2, space="PSUM") as ps_y,
            tc.tile_pool(name="pbuf", bufs=4) as pbuf,
            tc.tile_pool(name="vbuf", bufs=3) as vbuf,
            tc.tile_pool(name="small", bufs=2) as small,
            tc.tile_pool(name="ybuf", bufs=2) as ybuf,
            tc.tile_pool(name="ytbuf", bufs=1) as ytbuf,
            tc.tile_pool(name="obuf", bufs=2) as obuf,
        ):
            # ---- persistent SBUF state (wp/bp loads deferred: keep the
            # DMA queue head free for the x chunks feeding the first qkv) ----
            id_sb = persist.tile([128, HD], F32R)
            nc.sync.dma_start(id_sb, ident.ap())
            ones_sb = persist.tile([128, 128], F32R)
            nc.sync.dma_start(ones_sb, ones128.ap())
            wq_sb = persist.tile([128, NC_C, OSL], F32R)
            _wqr = wqT.rearrange("(n p) f -> p n f", p=128)
            nc.sync.dma_start(wq_sb[:, 0:2, :], _wqr[:, 0:2, :])
            nc.sync.dma_start(wq_sb[:, 2:NC_C, :], _wqr[:, 2:NC_C, :])
            bq_sb = persist.tile([128, OSL // 128, 1], FP32)
            nc.sync.dma_start(bq_sb, bq.rearrange("(n p) o -> p n o", p=128))
            wp_sb = persist.tile([128, NC_C, C], F32R)
            bp_sb = persist.tile([1, C], F32R)

            # qkv^T, permuted layout: o-tile 0 = Q^T, 1 = K^T, 2 = V^T;
            # head0 at partitions 0:64, head1 at 64:128
            qkvT = persist.tile([128, 3, BT], F32R)
            bb = persist.tile([128, C], FP32)

            a2a_in = [dram.tile([N_CORES, 128, TSH], F32R, name=f"a2ai{b}")
                      for b in range(B)]
            a2a_out = [dram.tile([N_CORES, 128, TSH], F32R, name=f"a2ao{b}")
                       for b in range(B)]
            yT_sb = ytbuf.tile([128, NC_C, TSL], F32R)

            # ---- emission helpers ----
            def emit_x_dma(tci):
                sl = slice(tci * TCH, (tci + 1) * TCH)
                xc = xin.tile([128, NC_C, TCH], F32R, tag="xc", name=f"xc{tci}")
                xr = xT[:, sl].rearrange("(n p) f -> p n f", p=128)
                nc.sync.dma_start(xc[:, 0:2, :], xr[:, 0:2, :])
                nc.sync.dma_start(xc[:, 2:NC_C, :], xr[:, 2:NC_C, :])
                return xc

            def emit_qkv_group(xc, tci, ot):
                sl = slice(tci * TCH, (tci + 1) * TCH)
                ps = ps_qkv.tile([128, TCH], FP32, tag="ps", name=f"qk{tci}{ot}")
                for ct in range(NC_C):
                    nc.tensor.matmul(
                        ps, lhsT=wq_sb[:, ct, ot * 128:(ot + 1) * 128],
                        rhs=xc[:, ct, :], start=(ct == 0), stop=(ct == NC_C - 1))
                nc.vector.tensor_scalar_add(qkvT[:, ot, sl], ps, bq_sb[:, ot, :])

            def emit_va(b, hl):
                hp = hl * HD
                boff = b * T
                v_ap = qkvT[hp:hp + HD, 2, boff:boff + T]
                va = vbuf.tile([128, NKT, HD + 1], F32R, tag="va",
                               name=f"va{b}{hl}")
                nc.sync.dma_start(va[:, :, HD], ones128.ap()[:, 0:NKT])
                for k in range(NKT):
                    tp = ps_qkv.tile([128, HD], F32R, tag="ps", name=f"tp{k}")
                    nc.tensor.transpose(
                        tp, in_=v_ap[:, k * 128:(k + 1) * 128],
                        identity=id_sb[hp:hp + HD, :])
                    nc.vector.tensor_copy(va[:, k, 0:HD], tp)
                return va

            def emit_chunk(b, hl, qp, va, prev_finish=None):
                """One attention pair unit: S^T/exp/PV over all kt for TWO
                512-token query chunks (one 1024-wide exp per kt tile).
                Normalization+staging is returned as a closure deferred into
                the next unit (keeps the PE stream from stalling on the DVE
                reciprocal at unit boundaries)."""
                hp = hl * HD
                boff = b * T
                q_ap = qkvT[hp:hp + HD, 0, boff:boff + T]
                k_ap = qkvT[hp:hp + HD, 1, boff:boff + T]
                W = 2 * QCH
                qsl = slice(qp * W, (qp + 1) * W)
                ypx = [ps_y.tile([128, QCH], FP32, tag="ypx",
                                 name=f"y{b}{hl}{qp}{h}") for h in range(2)]
                for k in range(NKT):
                    sps = ps_sp.tile([128, W], FP32, tag="sps", name=f"s{k}")
                    for h in range(2):
                        nc.tensor.matmul(
                            sps[:, h * QCH:(h + 1) * QCH],
                            lhsT=k_ap[:, k * 128:(k + 1) * 128],
                            rhs=q_ap[:, qp * W + h * QCH:
                                     qp * W + (h + 1) * QCH],
                            start=True, stop=True)
                    if k == 0 and prev_finish is not None:
                        prev_finish()
                        prev_finish = None
                    pt = pbuf.tile([128, W], F32R, tag="pt", name=f"p{k}")
                    nc.scalar.activation(
                        pt, sps, mybir.ActivationFunctionType.Exp, scale=SCALE)
                    for h in range(2):
                        nc.tensor.matmul(
                            ypx[h][0:HD + 1, :], lhsT=va[:, k, :],
                            rhs=pt[:, h * QCH:(h + 1) * QCH],
                            start=(k == 0), stop=(k == NKT - 1))
                if prev_finish is not None:
                    prev_finish()

                def finish():
                    # normalize by the denominators accumulated in
                    # partition HD of each accumulator
                    for h in range(2):
                        qc = 2 * qp + h
                        rec = small.tile([HD + 1, QCH], F32R, tag="rec",
                                         name="rec")
                        with nc.allow_low_precision(reason="f32r recip"):
                            nc.vector.reciprocal(rec[HD:HD + 1, :],
                                                 ypx[h][HD:HD + 1, :])
                        rbp = ps_sp.tile([HD, QCH], FP32, tag="sps",
                                         name="rbp")
                        nc.tensor.matmul(rbp, lhsT=ones_sb[HD:HD + 1, 0:HD],
                                         rhs=rec[HD:HD + 1, :],
                                         start=True, stop=True)
                        rb = small.tile([HD, QCH], FP32, tag="rb", name="rb")
                        nc.vector.tensor_copy(rb, rbp)
                        yc = ybuf.tile([HD, QCH], F32R, tag="yc", name="yc")
                        nc.vector.tensor_tensor(yc, ypx[h][0:HD, :], rb,
                                                op=mybir.AluOpType.mult)
                        # dest core j holds batch tokens [j*TSH, (j+1)*TSH)
                        for half in range(2):
                            j = 2 * qc + half
                            nc.sync.dma_start(
                                a2a_in[b][j, hp:hp + HD, :],
                                yc[:, half * TSH:(half + 1) * TSH])
                return finish

            def emit_a2a(b):
                if single_core or variant == "noccl":
                    nc.sync.dma_start(a2a_out[b].opt(), a2a_in[b].opt())
                else:
                    nc.gpsimd.collective_compute(
                        "AllToAll", mybir.AluOpType.bypass,
                        replica_groups=[list(range(N_CORES))],
                        ins=[a2a_in[b].opt()], outs=[a2a_out[b].opt()])

            def emit_yt_load(b):
                for ct in range(NC_C):
                    nc.sync.dma_start(
                        yT_sb[:, ct, b * TSH:(b + 1) * TSH], a2a_out[b][ct])

            def emit_proj(tt):
                osb = obuf.tile([128, C], FP32, tag="osb", name=f"o{tt}")
                for oc in range(C // 512):
                    osl = slice(oc * 512, (oc + 1) * 512)
                    ps = ps_qkv.tile([128, 512], FP32, tag="ps", name=f"op{tt}{oc}")
                    for ct in range(NC_C):
                        nc.tensor.matmul(
                            ps, lhsT=yT_sb[:, ct, tt * 128:(tt + 1) * 128],
                            rhs=wp_sb[:, ct, osl],
                            start=(ct == 0), stop=(ct == NC_C - 1))
                    nc.vector.tensor_add(osb[:, osl], ps, bb[:, osl])
                nc.sync.dma_start(out[tt * 128:(tt + 1) * 128, :], osb)

            # ---- emission schedule (software pipeline) ----
            def emit_wp_load():
                nc.sync.dma_start(
                    wp_sb, wpT.rearrange("(n p) f -> p n f", p=128))
                nc.sync.dma_start(bp_sb, bp.ap())

            def emit_bias_bcast():
                for oc in range(C // 512):
                    osl = slice(oc * 512, (oc + 1) * 512)
                    bbp = ps_qkv.tile([128, 512], FP32, tag="ps",
                                    name=f"bbp{oc}")
                    nc.tensor.matmul(bbp, lhsT=ones_sb[0:1, :],
                                     rhs=bp_sb[:, osl], start=True, stop=True)
                    nc.vector.tensor_copy(bb[:, osl], bbp)

            # PE prewarm: ~4us of dummy matmul activity releases the HAM
            # clock-gate before the first real qkv matmul arrives
            warm = ps_sp.tile([128, 128], FP32, tag="sps", name="warm")
            for wi in range(18):
                nc.tensor.matmul(warm, lhsT=ones_sb, rhs=ones_sb,
                                 start=(wi == 0), stop=(wi == 17))

            # batch-0 qkv
            for tci in range(4):
                xc = emit_x_dma(tci)
                for ot in range(3):
                    emit_qkv_group(xc, tci, ot)

            if variant == "qkv":
                for tci in range(4, 8):
                    xc = emit_x_dma(tci)
                    for ot in range(3):
                        emit_qkv_group(xc, tci, ot)
                raise _EndEmission()

            vas = {}
            vas[(0, 0)] = emit_va(0, 0)
            vas[(0, 1)] = emit_va(0, 1)

            # filler work interleaved into batch-0 attention chunk units:
            # batch-1 qkv, deferred weight loads, batch-1 V_aug builds
            state = {"xc": None}
            # (pe_cost, fn): DMA-only items cost 0 and don't consume a slot
            fillers = []
            for tci in range(4, 8):
                fillers.append((0, lambda t=tci: state.update(
                    xc=emit_x_dma(t))))
                for ot in range(3):
                    fillers.append((1, lambda t=tci, o=ot: emit_qkv_group(
                        state["xc"], t, o)))
            fillers.append((0, emit_wp_load))
            fillers.append((1, lambda: vas.update({(1, 0): emit_va(1, 0)})))
            fillers.append((1, lambda: vas.update({(1, 1): emit_va(1, 1)})))
            fillers.append((1, emit_bias_bcast))

            def pop_fillers(n):
                budget = n
                while fillers and budget > 0:
                    cost, f = fillers.pop(0)
                    f()
                    budget -= cost

            pending = None
            for hl in range(HPC):
                for qp in range(NQC // 2):
                    pending = emit_chunk(0, hl, qp, vas[(0, hl)], pending)
                    pop_fillers(4)
            while fillers:
                fillers.pop(0)[1]()
            pending()
            pending = None
            if variant == "attn_b0":
                raise _EndEmission()
            emit_a2a(0)

            # batch-1 attention with first-half projection interleaved
            for hl in range(HPC):
                for qp in range(NQC // 2):
                    pending = emit_chunk(1, hl, qp, vas[(1, hl)], pending)
                    if hl == 0 and qp == 1:
                        emit_yt_load(0)
                    elif hl == 1 and qp == 1:
                        emit_proj(0)
                        emit_proj(1)
            pending()
            if variant == "attn":
                raise _EndEmission()
            emit_a2a(1)
            emit_yt_load(1)
            emit_proj(2)
            emit_proj(3)

      except _EndEmission:
        pass
    nc.compile()
    return nc


_NC_CACHE = None


def _get_program():
    global _NC_CACHE
    if _NC_CACHE is None:
        _NC_CACHE = build_program()
    return _NC_CACHE


# permutation of the 384 local qkv rows: tile0 = [q_h0, q_h1],
# tile1 = [k_h0, k_h1], tile2 = [v_h0, v_h1]
def _local_perm():
    p = []
    for kind in range(3):            # q, k, v
        for hl in range(HPC):
            base = hl * 3 * HD + kind * HD
            p.extend(range(base, base + HD))
    return np.array(p, dtype=np.int64)


def make_in_maps(x, w_attn, b_attn, w_proj, b_proj):
    x = np.asarray(x, dtype=np.float32)
    w_attn = np.asarray(w_attn, dtype=np.float32)
    b_attn = np.asarray(b_attn, dtype=np.float32)
    w_proj = np.asarray(w_proj, dtype=np.float32)
    b_proj = np.asarray(b_proj, dtype=np.float32)

    xT = np.ascontiguousarray(x.reshape(BT, C).T)
    wpT = np.ascontiguousarray(w_proj.T)
    bp = np.ascontiguousarray(b_proj.reshape(1, C))
    ident = np.concatenate([np.eye(HD, dtype=np.float32)] * 2, axis=0)
    ones128 = np.ones((128, 128), dtype=np.float32)
    perm = _local_perm()

    in_maps = []
    for i in range(N_CORES):
        sl = slice(i * OSL, (i + 1) * OSL)
        w_slice = w_attn[sl][perm]
        b_slice = b_attn[sl][perm]
        in_maps.append({
            "xT": xT,
            "wqT": np.ascontiguousarray(w_slice.T),
            "bq": np.ascontiguousarray(b_slice.reshape(OSL, 1)),
            "wpT": wpT,
            "bp": bp,
            "ident": ident,
            "ones128": ones128,
        })
    return in_maps


def _assemble(shards):
    """shards[j] is [TSL, C]: rows 0:TSH = b0 tokens j*TSH.., rows
    TSH:2*TSH = b1 tokens j*TSH.. Reassemble into [B, T, C]."""
    out = np.empty((BT, C), dtype=np.float32)
    for j in range(N_CORES):
        out[j * TSH:(j + 1) * TSH] = shards[j][0:TSH]
        out[T + j * TSH:T + (j + 1) * TSH] = shards[j][TSH:2 * TSH]
    return out.reshape(B, T, C)


def kernel(x, w_attn, b_attn, w_proj, b_proj):
    nc = _get_program()
    in_maps = make_in_maps(x, w_attn, b_attn, w_proj, b_proj)

    if os.environ.get("BASS_KERNEL_SIM") == "1":
        from concourse.bass_interp import MultiCoreSim
        sim = MultiCoreSim(nc, num_cores=N_CORES,
                           num_workers=int(os.environ.get("BASS_SIM_WORKERS", "8")))
        for i in range(N_CORES):
            core = sim.cores[i]
            for k, v in in_maps[i].items():
                core.tensor(k)[:] = v
        sim.simulate(check_with_hw=False)
        shards = [np.array(sim.cores[i].tensor("out")) for i in range(N_CORES)]
    else:
        last_err = None
        shards = None
        for _attempt in range(3):
            try:
                res = run_bass_kernel_spmd(nc, in_maps,
                                           core_ids=list(range(N_CORES)))
                shards = [res.results[i]["out"] for i in range(N_CORES)]
                break
            except Exception as e:  # transient device wedge: retry
                last_err = e
        if shards is None:
            raise last_err

    return _assemble(shards)
